# revision 4
# baseline (speedup 1.0000x reference)
"""Entity-linking bilinear retrieval kernel for 8 TRN2 NeuronCores.

scores = (emb_a @ W) @ emb_b.T + b ; outputs (row max, row argmax, max > 0).

Sharding: emb_a rows split 8 ways (512 rows/core); W and emb_b replicated.
Each core computes its [512, 4096] score block on-device and reduces each
row to per-512-column-chunk (top-1 value, local argmax) pairs; the final
8-way combine across chunks runs on host in numpy (exact, first-occurrence
tie-break identical to jnp.argmax).

Device layout notes:
- All matmuls take pre-transposed operands, so the host ships emb_a.T
  slices and emb_b.T and W in natural layout.
- Step 1 computes A_T = (emb_a_loc @ W).T directly ([768, 512]), which is
  exactly the lhsT layout step 2 needs -> no on-device transposes at all.
- Row max/argmax uses the DVE MAX8/MAX_INDEX8 instructions straight out of
  PSUM, so score tiles are never copied to SBUF.
"""

import numpy as np

N, M, H = 4096, 4096, 768
NCORES = 8
NLOC = N // NCORES  # rows of emb_a per core
P = 128             # partitions
KT = H // P         # contraction tiles (6)
MT = NLOC // P      # output row tiles per core (4)
NTILE = 512         # matmul free-dim tile / argmax chunk
NT = M // NTILE     # column chunks (8)

_PROGRAM_CACHE: dict = {}
_RUNNER_CACHE: dict = {}


def _build_program(mm_dtype_name: str = "float32", reps: int = 1):
    from contextlib import ExitStack

    import concourse.mybir as mybir
    import concourse.tile as tile
    from concourse import bacc

    f32 = mybir.dt.float32
    u32 = mybir.dt.uint32
    mm_dt = getattr(mybir.dt, mm_dtype_name)

    nc = bacc.Bacc("TRN2", target_bir_lowering=False, debug=False)

    ea_t = nc.dram_tensor("ea_t", [H, NLOC], f32, kind="ExternalInput")
    w_d = nc.dram_tensor("w", [H, H], f32, kind="ExternalInput")
    eb_t = nc.dram_tensor("eb_t", [H, M], f32, kind="ExternalInput")
    vals_d = nc.dram_tensor("vals", [NLOC, NT, 8], f32, kind="ExternalOutput")
    idxs_d = nc.dram_tensor("idxs", [NLOC, NT, 8], u32, kind="ExternalOutput")

    def mm(ap):
        return ap if mm_dtype_name == "float32" else ap.bitcast(mm_dt)

    def emit_body(tc, ctx, consts, psum, outs):
        # [h1, h2] -> [p, kt, h2]; per-partition chunks stay contiguous
        w_sb = consts.tile([P, KT, H], f32, tag="w_sb", name="w_sb")
        nc.sync.dma_start(w_sb[:], w_d.ap().rearrange("(kt p) m -> p kt m", p=P))
        ea_sb = consts.tile([P, KT, NLOC], f32, tag="ea_sb", name="ea_sb")
        nc.sync.dma_start(ea_sb[:], ea_t.ap().rearrange("(kt p) n -> p kt n", p=P))

        # emb_b.T loaded per column chunk so step-2 compute can start
        # before the whole 12.6MB replica lands
        eb_sb = consts.tile([P, KT, M], f32, tag="eb_sb", name="eb_sb")
        for n in range(NT):
            nc.sync.dma_start(
                eb_sb[:, :, n * NTILE:(n + 1) * NTILE],
                eb_t.ap()[:, n * NTILE:(n + 1) * NTILE].rearrange(
                    "(kt p) m -> p kt m", p=P
                ),
            )

        # step 1: A_T[h2, i] = sum_h1 W[h1, h2] * emb_a_loc.T[h1, i]
        a_sb = consts.tile([P, KT, NLOC], f32, tag="a_sb", name="a_sb")
        for m_i in range(KT):
            pa = psum.tile([P, NLOC], f32, tag="pa", bufs=2, name="pa")
            for k in range(KT):
                nc.tensor.matmul(
                    pa[:],
                    mm(w_sb[:, k, m_i * P:(m_i + 1) * P]),
                    mm(ea_sb[:, k, :]),
                    start=(k == 0),
                    stop=(k == KT - 1),
                )
            nc.vector.tensor_copy(a_sb[:, m_i, :], pa[:])

        # step 2: scores chunk [128, 512] per (n, mi), then DVE top-8 +
        # argmax straight out of PSUM
        vals_sb = []
        idxs_sb = []
        for mi in range(MT):
            vt = outs.tile([P, NT, 8], f32, tag=f"vals{mi}", name=f"vals_sb{mi}")
            it = outs.tile([P, NT, 8], u32, tag=f"idxs{mi}", name=f"idxs_sb{mi}")
            vals_sb.append(vt)
            idxs_sb.append(it)

        for n in range(NT):
            for mi in range(MT):
                ps = psum.tile([P, NTILE], f32, tag="ps", bufs=4, name="ps")
                for k in range(KT):
                    nc.tensor.matmul(
                        ps[:],
                        mm(a_sb[:, k, mi * P:(mi + 1) * P]),
                        mm(eb_sb[:, k, n * NTILE:(n + 1) * NTILE]),
                        start=(k == 0),
                        stop=(k == KT - 1),
                    )
                nc.vector.max(vals_sb[mi][:, n, :], ps[:])
                nc.vector.max_index(idxs_sb[mi][:, n, :], vals_sb[mi][:, n, :], ps[:])

        for mi in range(MT):
            nc.sync.dma_start(vals_d.ap()[mi * P:(mi + 1) * P, :, :], vals_sb[mi][:])
            nc.sync.dma_start(idxs_d.ap()[mi * P:(mi + 1) * P, :, :], idxs_sb[mi][:])

    with tile.TileContext(nc) as tc:
        with ExitStack() as ctx:
            consts = ctx.enter_context(tc.tile_pool(name="consts", bufs=1))
            psum = ctx.enter_context(tc.tile_pool(name="psum", bufs=2, space="PSUM"))
            outs = ctx.enter_context(tc.tile_pool(name="outs", bufs=1))
            for _ in range(reps):
                emit_body(tc, ctx, consts, psum, outs)

    nc.compile()
    return nc


def _get_program(mm_dtype_name: str, reps: int = 1):
    key = (mm_dtype_name, reps)
    prog = _PROGRAM_CACHE.get(key)
    if prog is None:
        prog = _build_program(mm_dtype_name, reps)
        _PROGRAM_CACHE[key] = prog
    return prog


def _shard_inputs(emb_a, emb_b, W):
    eb_t = np.ascontiguousarray(emb_b.T)
    in_maps = []
    for c in range(NCORES):
        ea_t = np.ascontiguousarray(emb_a[c * NLOC:(c + 1) * NLOC].T)
        in_maps.append({"ea_t": ea_t, "w": W, "eb_t": eb_t})
    return in_maps


def _combine(results, b):
    best_list, idx_list = [], []
    rows = np.arange(NLOC)
    for c in range(NCORES):
        vals = results[c]["vals"]  # [NLOC, NT, 8] f32, per-chunk top8 desc
        idxs = results[c]["idxs"]  # [NLOC, NT, 8] u32, matching indices
        ctop = vals[:, :, 0]                       # [NLOC, NT] chunk maxima
        carg = idxs[:, :, 0].astype(np.int64)      # [NLOC, NT] local argmax
        csel = np.argmax(ctop, axis=1)             # first-occurrence, like jnp
        best_list.append(ctop[rows, csel])
        idx_list.append(csel * NTILE + carg[rows, csel])

    best_scores = (np.concatenate(best_list) + b[0]).astype(np.float32)
    best_idx = np.concatenate(idx_list).astype(np.int32)
    valid = best_scores > np.float32(0.0)
    return best_scores, best_idx, valid


def _run(emb_a, emb_b, W, b, mm_dtype="float32", trace=False):
    from concourse.bass_utils import run_bass_kernel_spmd

    nc = _get_program(mm_dtype)
    in_maps = _shard_inputs(emb_a, emb_b, W)
    res = run_bass_kernel_spmd(nc, in_maps, list(range(NCORES)), trace=trace)
    return _combine(res.results, b), res


def kernel(**inputs):
    emb_a = np.asarray(inputs["emb_a"], dtype=np.float32)
    emb_b = np.asarray(inputs["emb_b"], dtype=np.float32)
    W = np.asarray(inputs["W"], dtype=np.float32)
    b = np.asarray(inputs["b"], dtype=np.float32)
    outs, _ = _run(emb_a, emb_b, W, b)
    return outs


# ----------------------------------------------------------------------------
# Benchmark path: cached jitted callable (device inputs pre-placed) so the
# same program can be invoked repeatedly with low overhead; device time is
# obtained by differencing reps=1 vs reps=K unrolled program variants.
# ----------------------------------------------------------------------------

def _make_runner(mm_dtype: str, reps: int, in_maps):
    import jax
    import jax.numpy as jnp
    from jax.sharding import Mesh, NamedSharding, PartitionSpec
    from jax.experimental.shard_map import shard_map

    import concourse.mybir as mybir
    from concourse import bass2jax

    nc = _get_program(mm_dtype, reps)
    bass2jax.install_neuronx_cc_hook()

    partition_name = nc.partition_id_tensor.name if nc.partition_id_tensor else None
    in_names, out_names, out_avals, zero_outs = [], [], [], []
    for alloc in nc.m.functions[0].allocations:
        if not isinstance(alloc, mybir.MemoryLocationSet):
            continue
        name = alloc.memorylocations[0].name
        if alloc.kind == "ExternalInput":
            if name != partition_name:
                in_names.append(name)
        elif alloc.kind == "ExternalOutput":
            out_names.append(name)
            shape = tuple(alloc.tensor_shape)
            dtype = mybir.dt.np(alloc.dtype)
            out_avals.append(jax.core.ShapedArray(shape, dtype))
            zero_outs.append(np.zeros(shape, dtype))
    n_params = len(in_names)
    n_outs = len(out_avals)
    all_in_names = list(in_names) + list(out_names)
    if partition_name is not None:
        all_in_names.append(partition_name)

    def _body(*args):
        operands = list(args)
        if partition_name is not None:
            operands.append(bass2jax.partition_id_tensor())
        outs = bass2jax._bass_exec_p.bind(
            *operands,
            out_avals=tuple(out_avals),
            in_names=tuple(all_in_names),
            out_names=tuple(out_names),
            lowering_input_output_aliases=(),
            sim_require_finite=True,
            sim_require_nnan=True,
            nc=nc,
        )
        return tuple(outs)

    devices = jax.devices()[:NCORES]
    mesh = Mesh(np.asarray(devices), ("core",))
    in_specs = (PartitionSpec("core"),) * (n_params + n_outs)
    out_specs = (PartitionSpec("core"),) * n_outs
    donate = tuple(range(n_params, n_params + n_outs))
    sharded = jax.jit(
        shard_map(_body, mesh=mesh, in_specs=in_specs, out_specs=out_specs,
                  check_rep=False),
        donate_argnums=donate,
        keep_unused=True,
    )

    sh = NamedSharding(mesh, PartitionSpec("core"))
    concat_in = [
        jax.device_put(
            np.concatenate([np.asarray(in_maps[c][nm]) for c in range(NCORES)], axis=0),
            sh,
        )
        for nm in in_names
    ]
    zero_shapes = [(NCORES * z.shape[0], *z.shape[1:]) for z in zero_outs]
    zero_dtypes = [z.dtype for z in zero_outs]

    def call():
        zeros = [
            jax.device_put(np.zeros(s, d), sh)
            for s, d in zip(zero_shapes, zero_dtypes)
        ]
        outs = sharded(*concat_in, *zeros)
        jax.block_until_ready(outs)
        return outs

    return call, out_names, out_avals


def bench_device_time(emb_a, emb_b, W, mm_dtype="float32", reps_hi=9, calls=10):
    """Returns (t_lo_med, t_hi_med, per_rep_ns) using reps differencing."""
    import time

    in_maps = _shard_inputs(emb_a, emb_b, W)
    times = {}
    for reps in (1, reps_hi):
        key = (mm_dtype, reps)
        if key not in _RUNNER_CACHE:
            _RUNNER_CACHE[key] = _make_runner(mm_dtype, reps, in_maps)
        call, _, _ = _RUNNER_CACHE[key]
        call()  # warm/compile
        samples = []
        for _ in range(calls):
            t0 = time.perf_counter()
            call()
            samples.append(time.perf_counter() - t0)
        samples.sort()
        times[reps] = samples
    lo = np.median(times[1])
    hi = np.median(times[reps_hi])
    per_rep_ns = (hi - lo) / (reps_hi - 1) * 1e9
    return lo, hi, per_rep_ns
